# revision 16
# baseline (speedup 1.0000x reference)
"""Multi-head attention with dropout on 8 Trainium2 NeuronCores.

Problem: B=2, H=16, S=2048, D=64 attention where x2 serves as both keys and
values, softmax over keys, then a deterministic jax dropout mask (key 42,
p=0.1) applied to the probabilities before the value matmul.

Sharding: the 32 (b,h) pairs are split 4-per-core across 8 cores (pure head
parallelism, no collectives).

Device formulation (per head): scores are computed TRANSPOSED, St[k,q] =
sum_d K[k,d]Q[q,d], by matmul(lhsT=K^T chunk, rhs=Q^T). Softmax max-
subtraction is skipped (scores ~ N(0,1), exp is safe in fp32). The exp runs
on the scalar engine with the 1/sqrt(64) fold into its scale. The dropout
mask (shipped from host as bf16 {0,1}, transposed to [k,q]) is applied by
the vector engine. The second matmul uses V'=[V|ones] as stationary weights
twice per k-chunk: once against masked exp (rows 0..63 accumulate the
unnormalized output, transposed [d,q]) and once against unmasked exp (row 64
accumulates the softmax denominator). Normalization by 1/(0.9*sum) and the
final [d,q]->[q,d] transpose happen on host during the gather.
"""

import sys

sys.path.insert(0, "/opt/trn_rl_repo")

import numpy as np
import ml_dtypes

import concourse.bass as bass
import concourse.mybir as mybir
import concourse.tile as tile
from concourse import bacc
from concourse.bass_utils import run_bass_kernel_spmd

B, H, S, D = 2, 16, 2048, 64
N_CORES = 8
HEADS_PER_CORE = (B * H) // N_CORES  # 4
DROP_P = 0.1

QB = 1024  # q columns per activation/vector instruction
MMN = 512  # q columns per matmul instruction (PSUM bank limit, fp32 out)
KT = 128  # k rows per score tile (matmul output partitions)
N_KT = S // KT  # 16
N_QB = S // QB  # 2

BF16 = mybir.dt.bfloat16
F32 = mybir.dt.float32

LAST_RESULTS = None  # BassKernelResults of the most recent run (for test.py)


def build_program():
    nc = bacc.Bacc()

    qt = nc.dram_tensor("qt", [HEADS_PER_CORE, D, S], BF16, kind="ExternalInput")
    kt = nc.dram_tensor("kt", [HEADS_PER_CORE, D, S], BF16, kind="ExternalInput")
    v = nc.dram_tensor("v", [HEADS_PER_CORE, S, D], BF16, kind="ExternalInput")
    # mask pre-tiled on host: [h, k-chunk, qb, 128, QB] so each tile DMA is
    # one fully contiguous 256KB read
    mask = nc.dram_tensor(
        "mask", [HEADS_PER_CORE, N_KT, N_QB, KT, QB], BF16, kind="ExternalInput"
    )
    outu = nc.dram_tensor("outu", [HEADS_PER_CORE, D, S], F32, kind="ExternalOutput")
    sums = nc.dram_tensor("sums", [HEADS_PER_CORE, S], F32, kind="ExternalOutput")

    with tile.TileContext(nc) as tc:
        with (
            tc.tile_pool(name="qkpool", bufs=2) as qkpool,
            tc.tile_pool(name="vpool", bufs=2) as vpool,
            tc.tile_pool(name="epool", bufs=10) as epool,
            tc.tile_pool(name="empool", bufs=6) as empool,
            tc.tile_pool(name="mpool", bufs=12) as mpool,
            tc.tile_pool(name="opool", bufs=2) as opool,
            tc.tile_pool(name="stpool", bufs=2, space=bass.MemorySpace.PSUM) as stpool,
            tc.tile_pool(name="accpool", bufs=1, space=bass.MemorySpace.PSUM) as accpool,
            tc.tile_pool(name="sumpool", bufs=1, space=bass.MemorySpace.PSUM) as sumpool,
        ):
            for h in range(HEADS_PER_CORE):
                # K^T split in halves and Q^T loaded per qb so the first
                # matmuls only wait on ~0.5MB of DMA, not the whole head's
                # inputs (tile deps are whole-tile).
                kt_halves = []
                for kh in range(2):
                    kt_sb = qkpool.tile([D, S // 2], BF16, tag=f"kt{kh}")
                    nc.sync.dma_start(kt_sb[:], kt[h][:, kh * (S // 2) : (kh + 1) * (S // 2)])
                    kt_halves.append(kt_sb)
                vt_sb = None

                for qb in range(N_QB):
                    qt_sb = qkpool.tile([D, QB], BF16, tag="qt")
                    nc.sync.dma_start(qt_sb[:], qt[h][:, qb * QB : (qb + 1) * QB])
                    out_acc = accpool.tile([D + 1, QB], F32, tag="acc")
                    sum_acc = sumpool.tile([D + 1, QB], F32, tag="acc2")
                    equads = []  # e tiles of the in-flight group of 4 k-chunks
                    for k in range(N_KT):
                        # PSUM bank limit: matmul N<=512 fp32 out, so each
                        # QB-wide tile is produced/consumed by the tensor
                        # engine in MMN-column slices while ACT/DVE see the
                        # full QB width in one instruction.
                        st = stpool.tile([KT, QB], F32, tag="st")
                        kth = kt_halves[k // (N_KT // 2)]
                        kk = k % (N_KT // 2)
                        for s in range(QB // MMN):
                            nc.tensor.matmul(
                                st[:, s * MMN : (s + 1) * MMN],
                                kth[:, kk * KT : (kk + 1) * KT],
                                qt_sb[:, s * MMN : (s + 1) * MMN],
                                start=True,
                                stop=True,
                            )
                        e = epool.tile([KT, QB], BF16, tag="e")
                        nc.scalar.activation(
                            e[:], st[:], mybir.ActivationFunctionType.Exp, scale=0.125
                        )
                        m = mpool.tile([KT, QB], BF16, tag="m")
                        nc.sync.dma_start(m[:], mask[h, k, qb])
                        if vt_sb is None:
                            # V' = [V | ones] per k-chunk, loaded after the
                            # first score tile's inputs are already in flight
                            vt_sb = vpool.tile([KT, N_KT, D + 1], BF16, tag="vt")
                            nc.sync.dma_start(
                                vt_sb[:, :, 0:D],
                                v[h].rearrange("(n p) d -> p n d", p=KT),
                            )
                            nc.gpsimd.memset(vt_sb[:, :, D : D + 1], 1.0)
                        em = empool.tile([KT, QB], BF16, tag="em")
                        nc.vector.tensor_mul(em[:], e[:], m[:])
                        for s in range(QB // MMN):
                            nc.tensor.matmul(
                                out_acc[:, s * MMN : (s + 1) * MMN],
                                vt_sb[:, k, :],
                                em[:, s * MMN : (s + 1) * MMN],
                                start=(k == 0),
                                stop=(k == N_KT - 1),
                            )
                        equads.append(e)
                        if len(equads) == 8:
                            # Softmax denominator: instead of a 3rd full PE
                            # stream (ones-row matmul per k-chunk), tree-add
                            # the 4 exp tiles on the DVE (bf16 2x mode, spare
                            # capacity) and feed one quad matmul that rides
                            # the ALREADY-LOADED V' weights of this k — its
                            # ones row accumulates the exact unmasked sums
                            # (rows 0..63 are junk); no extra weight swap.
                            eps = []
                            for p in range(4):
                                ep = epool.tile([KT, QB], BF16, tag="ep")
                                nc.vector.tensor_add(
                                    ep[:], equads[2 * p][:], equads[2 * p + 1][:]
                                )
                                eps.append(ep)
                            eq0 = epool.tile([KT, QB], BF16, tag="eq")
                            nc.vector.tensor_add(eq0[:], eps[0][:], eps[1][:])
                            eq1 = epool.tile([KT, QB], BF16, tag="eq")
                            nc.vector.tensor_add(eq1[:], eps[2][:], eps[3][:])
                            eq = epool.tile([KT, QB], BF16, tag="eq8")
                            nc.vector.tensor_add(eq[:], eq0[:], eq1[:])
                            j = k // 8
                            for s in range(QB // MMN):
                                nc.tensor.matmul(
                                    sum_acc[:, s * MMN : (s + 1) * MMN],
                                    vt_sb[:, k, :],
                                    eq[:, s * MMN : (s + 1) * MMN],
                                    start=(j == 0),
                                    stop=(j == N_KT // 8 - 1),
                                )
                            equads = []
                    out_sb = opool.tile([D, QB], F32, tag="out")
                    nc.vector.tensor_copy(out_sb[:], out_acc[0:D, :])
                    # PSUM reads must start at partition 0 (offset-64 reads
                    # crash the device), so copy the full [65,QB] accumulator
                    # (same cost — engine time scales with free size) and DMA
                    # the sums row out of SBUF instead. On ScalarE so both
                    # accumulator drains run in parallel (shorter PE stall at
                    # the qb boundary — PSUM accumulators have no double
                    # buffer to spare).
                    sums_sb = opool.tile([D + 1, QB], F32, tag="sums")
                    nc.vector.tensor_copy(sums_sb[:], sum_acc[:])
                    nc.sync.dma_start(outu[h, :, qb * QB : (qb + 1) * QB], out_sb[:])
                    nc.sync.dma_start(
                        sums[h : h + 1, qb * QB : (qb + 1) * QB],
                        sums_sb[D : D + 1, :],
                    )

    nc.finalize()
    return nc


_NC_CACHE = None


def _get_nc():
    global _NC_CACHE
    if _NC_CACHE is None:
        _NC_CACHE = build_program()
    return _NC_CACHE


def _dropout_mask_t():
    """keep mask from the reference's fixed jax key, transposed to [bh,k,q]
    and pre-tiled to [bh, k-chunk, qb, 128, QB] bf16 {0,1} so each device
    tile is one contiguous 256KB DMA. Computed on CPU; threefry is
    platform-deterministic."""
    import jax

    cpu = jax.devices("cpu")[0]
    with jax.default_device(cpu):
        keep = jax.random.bernoulli(jax.random.key(42), 1.0 - DROP_P, (B, H, S, S))
        mask_t = jax.numpy.transpose(keep, (0, 1, 3, 2)).astype(jax.numpy.bfloat16)
        mask_t = jax.numpy.reshape(mask_t, (B * H, N_KT, KT, N_QB, QB))
        mask_t = jax.numpy.transpose(mask_t, (0, 1, 3, 2, 4))
        mask_t = np.ascontiguousarray(np.asarray(mask_t))
    return mask_t


def kernel(x1: np.ndarray, x2: np.ndarray, _trace: bool = False) -> np.ndarray:
    global LAST_RESULTS
    nc = _get_nc()

    bh = B * H
    x1f = np.asarray(x1, dtype=np.float32).reshape(bh, S, D)
    x2f = np.asarray(x2, dtype=np.float32).reshape(bh, S, D)
    qt_all = np.ascontiguousarray(x1f.transpose(0, 2, 1)).astype(ml_dtypes.bfloat16)
    kt_all = np.ascontiguousarray(x2f.transpose(0, 2, 1)).astype(ml_dtypes.bfloat16)
    v_all = x2f.astype(ml_dtypes.bfloat16)
    mask_all = _dropout_mask_t()

    in_maps = []
    for c in range(N_CORES):
        sl = slice(c * HEADS_PER_CORE, (c + 1) * HEADS_PER_CORE)
        in_maps.append(
            {
                "qt": qt_all[sl],
                "kt": kt_all[sl],
                "v": v_all[sl],
                "mask": mask_all[sl],
            }
        )

    res = run_bass_kernel_spmd(nc, in_maps, core_ids=list(range(N_CORES)), trace=_trace)
    LAST_RESULTS = res

    outu = np.concatenate([r["outu"] for r in res.results], axis=0)  # [32, D, S]
    sums = np.concatenate([r["sums"] for r in res.results], axis=0)  # [32, S]

    denom = (1.0 - DROP_P) * sums  # [32, S] (per q)
    out = outu / denom[:, None, :]  # [32, D, S]
    out = out.transpose(0, 2, 1).reshape(B, H, S, D)
    return np.ascontiguousarray(out.astype(np.float32))


# revision 17
# speedup vs baseline: 1.2192x; 1.2192x over previous
"""Multi-head attention with dropout on 8 Trainium2 NeuronCores.

Problem: B=2, H=16, S=2048, D=64 attention where x2 serves as both keys and
values, softmax over keys, then a deterministic jax dropout mask (key 42,
p=0.1) applied to the probabilities before the value matmul.

Sharding: the 32 (b,h) pairs are split 4-per-core across 8 cores (pure head
parallelism, no collectives).

Device formulation (per head): scores are computed TRANSPOSED, St[k,q] =
sum_d K[k,d]Q[q,d], by matmul(lhsT=K^T chunk, rhs=Q^T). Softmax max-
subtraction is skipped (scores ~ N(0,1), exp is safe in fp32). The exp runs
on the scalar engine with the 1/sqrt(64) fold into its scale. The dropout
mask (shipped from host as bf16 {0,1}, transposed to [k,q]) is applied by
the vector engine. The second matmul uses V'=[V|ones] as stationary weights
twice per k-chunk: once against masked exp (rows 0..63 accumulate the
unnormalized output, transposed [d,q]) and once against unmasked exp (row 64
accumulates the softmax denominator). Normalization by 1/(0.9*sum) and the
final [d,q]->[q,d] transpose happen on host during the gather.
"""

import sys

sys.path.insert(0, "/opt/trn_rl_repo")

import numpy as np
import ml_dtypes

import concourse.bass as bass
import concourse.mybir as mybir
import concourse.tile as tile
from concourse import bacc
from concourse.bass_utils import run_bass_kernel_spmd

B, H, S, D = 2, 16, 2048, 64
N_CORES = 8
HEADS_PER_CORE = (B * H) // N_CORES  # 4
DROP_P = 0.1

QB = 1024  # q columns per activation/vector instruction
MMN = 512  # q columns per matmul instruction (PSUM bank limit, fp32 out)
KT = 128  # k rows per score tile (matmul output partitions)
N_KT = S // KT  # 16
N_QB = S // QB  # 2

BF16 = mybir.dt.bfloat16
F32 = mybir.dt.float32

LAST_RESULTS = None  # BassKernelResults of the most recent run (for test.py)


def build_program():
    nc = bacc.Bacc()

    qt = nc.dram_tensor("qt", [HEADS_PER_CORE, D, S], BF16, kind="ExternalInput")
    kt = nc.dram_tensor("kt", [HEADS_PER_CORE, D, S], BF16, kind="ExternalInput")
    v = nc.dram_tensor("v", [HEADS_PER_CORE, S, D], BF16, kind="ExternalInput")
    # mask pre-tiled on host: [h, k-chunk, qb, 128, QB] so each tile DMA is
    # one fully contiguous 256KB read
    mask = nc.dram_tensor(
        "mask", [HEADS_PER_CORE, N_KT, N_QB, KT, QB], BF16, kind="ExternalInput"
    )
    outu = nc.dram_tensor("outu", [HEADS_PER_CORE, D, S], F32, kind="ExternalOutput")
    sums = nc.dram_tensor("sums", [HEADS_PER_CORE, S], F32, kind="ExternalOutput")

    with tile.TileContext(nc) as tc:
        with (
            tc.tile_pool(name="qkpool", bufs=2) as qkpool,
            tc.tile_pool(name="vpool", bufs=2) as vpool,
            tc.tile_pool(name="epool", bufs=6) as epool,
            tc.tile_pool(name="empool", bufs=6) as empool,
            tc.tile_pool(name="mpool", bufs=12) as mpool,
            tc.tile_pool(name="opool", bufs=2) as opool,
            tc.tile_pool(name="stpool", bufs=2, space=bass.MemorySpace.PSUM) as stpool,
            tc.tile_pool(name="accpool", bufs=1, space=bass.MemorySpace.PSUM) as accpool,
            tc.tile_pool(name="sumpool", bufs=1, space=bass.MemorySpace.PSUM) as sumpool,
        ):
            for h in range(HEADS_PER_CORE):
                # K^T split in halves and Q^T loaded per qb so the first
                # matmuls only wait on ~0.5MB of DMA, not the whole head's
                # inputs (tile deps are whole-tile).
                kt_halves = []
                for kh in range(2):
                    kt_sb = qkpool.tile([D, S // 2], BF16, tag=f"kt{kh}")
                    nc.sync.dma_start(kt_sb[:], kt[h][:, kh * (S // 2) : (kh + 1) * (S // 2)])
                    kt_halves.append(kt_sb)
                vt_sb = None

                for qb in range(N_QB):
                    qt_sb = qkpool.tile([D, QB], BF16, tag="qt")
                    nc.sync.dma_start(qt_sb[:], qt[h][:, qb * QB : (qb + 1) * QB])
                    out_acc = accpool.tile([D + 1, QB], F32, tag="acc")
                    sum_acc = sumpool.tile([D + 1, QB], F32, tag="acc2")
                    equads = []  # e tiles of the in-flight group of 4 k-chunks
                    for k in range(N_KT):
                        # PSUM bank limit: matmul N<=512 fp32 out, so each
                        # QB-wide tile is produced/consumed by the tensor
                        # engine in MMN-column slices while ACT/DVE see the
                        # full QB width in one instruction.
                        st = stpool.tile([KT, QB], F32, tag="st")
                        kth = kt_halves[k // (N_KT // 2)]
                        kk = k % (N_KT // 2)
                        for s in range(QB // MMN):
                            nc.tensor.matmul(
                                st[:, s * MMN : (s + 1) * MMN],
                                kth[:, kk * KT : (kk + 1) * KT],
                                qt_sb[:, s * MMN : (s + 1) * MMN],
                                start=True,
                                stop=True,
                            )
                        e = epool.tile([KT, QB], BF16, tag="e")
                        nc.scalar.activation(
                            e[:], st[:], mybir.ActivationFunctionType.Exp, scale=0.125
                        )
                        m = mpool.tile([KT, QB], BF16, tag="m")
                        nc.sync.dma_start(m[:], mask[h, k, qb])
                        if vt_sb is None:
                            # V' = [V | ones] per k-chunk, loaded after the
                            # first score tile's inputs are already in flight
                            vt_sb = vpool.tile([KT, N_KT, D + 1], BF16, tag="vt")
                            nc.sync.dma_start(
                                vt_sb[:, :, 0:D],
                                v[h].rearrange("(n p) d -> p n d", p=KT),
                            )
                            nc.gpsimd.memset(vt_sb[:, :, D : D + 1], 1.0)
                        em = empool.tile([KT, QB], BF16, tag="em")
                        nc.vector.tensor_mul(em[:], e[:], m[:])
                        for s in range(QB // MMN):
                            nc.tensor.matmul(
                                out_acc[:, s * MMN : (s + 1) * MMN],
                                vt_sb[:, k, :],
                                em[:, s * MMN : (s + 1) * MMN],
                                start=(k == 0),
                                stop=(k == N_KT - 1),
                            )
                        equads.append(e)
                        if len(equads) == 4:
                            # Softmax denominator: instead of a 3rd full PE
                            # stream (ones-row matmul per k-chunk), tree-add
                            # the 4 exp tiles on the DVE (bf16 2x mode, spare
                            # capacity) and feed one quad matmul that rides
                            # the ALREADY-LOADED V' weights of this k — its
                            # ones row accumulates the exact unmasked sums
                            # (rows 0..63 are junk); no extra weight swap.
                            ep0 = epool.tile([KT, QB], BF16, tag="ep")
                            nc.vector.tensor_add(ep0[:], equads[0][:], equads[1][:])
                            ep1 = epool.tile([KT, QB], BF16, tag="ep")
                            nc.vector.tensor_add(ep1[:], equads[2][:], equads[3][:])
                            eq = epool.tile([KT, QB], BF16, tag="eq")
                            nc.vector.tensor_add(eq[:], ep0[:], ep1[:])
                            j = k // 4
                            for s in range(QB // MMN):
                                nc.tensor.matmul(
                                    sum_acc[:, s * MMN : (s + 1) * MMN],
                                    vt_sb[:, k, :],
                                    eq[:, s * MMN : (s + 1) * MMN],
                                    start=(j == 0),
                                    stop=(j == N_KT // 4 - 1),
                                )
                            equads = []
                    out_sb = opool.tile([D, QB], F32, tag="out")
                    nc.vector.tensor_copy(out_sb[:], out_acc[0:D, :])
                    # PSUM reads must start at partition 0 (offset-64 reads
                    # crash the device), so copy the full [65,QB] accumulator
                    # (same cost — engine time scales with free size) and DMA
                    # the sums row out of SBUF instead. On ScalarE so both
                    # accumulator drains run in parallel (shorter PE stall at
                    # the qb boundary — PSUM accumulators have no double
                    # buffer to spare).
                    sums_sb = opool.tile([D + 1, QB], F32, tag="sums")
                    nc.vector.tensor_copy(sums_sb[:], sum_acc[:])
                    nc.sync.dma_start(outu[h, :, qb * QB : (qb + 1) * QB], out_sb[:])
                    nc.sync.dma_start(
                        sums[h : h + 1, qb * QB : (qb + 1) * QB],
                        sums_sb[D : D + 1, :],
                    )

    nc.finalize()
    return nc


_NC_CACHE = None


def _get_nc():
    global _NC_CACHE
    if _NC_CACHE is None:
        _NC_CACHE = build_program()
    return _NC_CACHE


def _dropout_mask_t():
    """keep mask from the reference's fixed jax key, transposed to [bh,k,q]
    and pre-tiled to [bh, k-chunk, qb, 128, QB] bf16 {0,1} so each device
    tile is one contiguous 256KB DMA. Computed on CPU; threefry is
    platform-deterministic."""
    import jax

    cpu = jax.devices("cpu")[0]
    with jax.default_device(cpu):
        keep = jax.random.bernoulli(jax.random.key(42), 1.0 - DROP_P, (B, H, S, S))
        mask_t = jax.numpy.transpose(keep, (0, 1, 3, 2)).astype(jax.numpy.bfloat16)
        mask_t = jax.numpy.reshape(mask_t, (B * H, N_KT, KT, N_QB, QB))
        mask_t = jax.numpy.transpose(mask_t, (0, 1, 3, 2, 4))
        mask_t = np.ascontiguousarray(np.asarray(mask_t))
    return mask_t


def kernel(x1: np.ndarray, x2: np.ndarray, _trace: bool = False) -> np.ndarray:
    global LAST_RESULTS
    nc = _get_nc()

    bh = B * H
    x1f = np.asarray(x1, dtype=np.float32).reshape(bh, S, D)
    x2f = np.asarray(x2, dtype=np.float32).reshape(bh, S, D)
    qt_all = np.ascontiguousarray(x1f.transpose(0, 2, 1)).astype(ml_dtypes.bfloat16)
    kt_all = np.ascontiguousarray(x2f.transpose(0, 2, 1)).astype(ml_dtypes.bfloat16)
    v_all = x2f.astype(ml_dtypes.bfloat16)
    mask_all = _dropout_mask_t()

    in_maps = []
    for c in range(N_CORES):
        sl = slice(c * HEADS_PER_CORE, (c + 1) * HEADS_PER_CORE)
        in_maps.append(
            {
                "qt": qt_all[sl],
                "kt": kt_all[sl],
                "v": v_all[sl],
                "mask": mask_all[sl],
            }
        )

    res = run_bass_kernel_spmd(nc, in_maps, core_ids=list(range(N_CORES)), trace=_trace)
    LAST_RESULTS = res

    outu = np.concatenate([r["outu"] for r in res.results], axis=0)  # [32, D, S]
    sums = np.concatenate([r["sums"] for r in res.results], axis=0)  # [32, S]

    denom = (1.0 - DROP_P) * sums  # [32, S] (per q)
    out = outu / denom[:, None, :]  # [32, D, S]
    out = out.transpose(0, 2, 1).reshape(B, H, S, D)
    return np.ascontiguousarray(out.astype(np.float32))


# revision 18
# speedup vs baseline: 1.2353x; 1.0131x over previous
"""Multi-head attention with dropout on 8 Trainium2 NeuronCores.

Problem: B=2, H=16, S=2048, D=64 attention where x2 serves as both keys and
values, softmax over keys, then a deterministic jax dropout mask (key 42,
p=0.1) applied to the probabilities before the value matmul.

Sharding: the 32 (b,h) pairs are split 4-per-core across 8 cores (pure head
parallelism, no collectives).

Device formulation (per head): scores are computed TRANSPOSED, St[k,q] =
sum_d K[k,d]Q[q,d], by matmul(lhsT=K^T chunk, rhs=Q^T). Softmax max-
subtraction is skipped (scores ~ N(0,1), exp is safe in fp32). The exp runs
on the scalar engine with the 1/sqrt(64) fold into its scale. The dropout
mask (shipped from host as bf16 {0,1}, transposed to [k,q]) is applied by
the vector engine. The second matmul uses V'=[V|ones] as stationary weights
twice per k-chunk: once against masked exp (rows 0..63 accumulate the
unnormalized output, transposed [d,q]) and once against unmasked exp (row 64
accumulates the softmax denominator). Normalization by 1/(0.9*sum) and the
final [d,q]->[q,d] transpose happen on host during the gather.
"""

import sys

sys.path.insert(0, "/opt/trn_rl_repo")

import numpy as np
import ml_dtypes

import concourse.bass as bass
import concourse.mybir as mybir
import concourse.tile as tile
from concourse import bacc
from concourse.bass_utils import run_bass_kernel_spmd

B, H, S, D = 2, 16, 2048, 64
N_CORES = 8
HEADS_PER_CORE = (B * H) // N_CORES  # 4
DROP_P = 0.1

QB = 1024  # q columns per activation/vector instruction
MMN = 512  # q columns per matmul instruction (PSUM bank limit, fp32 out)
KT = 128  # k rows per score tile (matmul output partitions)
N_KT = S // KT  # 16
N_QB = S // QB  # 2

BF16 = mybir.dt.bfloat16
F32 = mybir.dt.float32

LAST_RESULTS = None  # BassKernelResults of the most recent run (for test.py)


def build_program():
    nc = bacc.Bacc()

    qt = nc.dram_tensor("qt", [HEADS_PER_CORE, D, S], BF16, kind="ExternalInput")
    kt = nc.dram_tensor("kt", [HEADS_PER_CORE, D, S], BF16, kind="ExternalInput")
    v = nc.dram_tensor("v", [HEADS_PER_CORE, S, D], BF16, kind="ExternalInput")
    # mask pre-tiled on host: [h, k-chunk, qb, 128, QB] so each tile DMA is
    # one fully contiguous 256KB read
    mask = nc.dram_tensor(
        "mask", [HEADS_PER_CORE, N_KT, N_QB, KT, QB], BF16, kind="ExternalInput"
    )
    outu = nc.dram_tensor("outu", [HEADS_PER_CORE, D, S], F32, kind="ExternalOutput")
    sums = nc.dram_tensor("sums", [HEADS_PER_CORE, S], F32, kind="ExternalOutput")

    with tile.TileContext(nc) as tc:
        with (
            tc.tile_pool(name="qkpool", bufs=2) as qkpool,
            tc.tile_pool(name="vpool", bufs=2) as vpool,
            tc.tile_pool(name="epool", bufs=6) as epool,
            tc.tile_pool(name="empool", bufs=6) as empool,
            tc.tile_pool(name="mpool", bufs=12) as mpool,
            tc.tile_pool(name="opool", bufs=2) as opool,
            tc.tile_pool(name="stpool", bufs=2, space=bass.MemorySpace.PSUM) as stpool,
            tc.tile_pool(name="accpool", bufs=1, space=bass.MemorySpace.PSUM) as accpool,
            tc.tile_pool(name="sumpool", bufs=1, space=bass.MemorySpace.PSUM) as sumpool,
        ):
            for h in range(HEADS_PER_CORE):
                # K^T split in halves and Q^T loaded per qb so the first
                # matmuls only wait on ~0.5MB of DMA, not the whole head's
                # inputs (tile deps are whole-tile).
                kt_halves = []
                for kh in range(2):
                    kt_sb = qkpool.tile([D, S // 2], BF16, tag=f"kt{kh}")
                    nc.sync.dma_start(kt_sb[:], kt[h][:, kh * (S // 2) : (kh + 1) * (S // 2)])
                    kt_halves.append(kt_sb)
                vt_sb = None

                for qb in range(N_QB):
                    qt_sb = qkpool.tile([D, QB], BF16, tag="qt")
                    nc.sync.dma_start(qt_sb[:], qt[h][:, qb * QB : (qb + 1) * QB])
                    out_acc = accpool.tile([D + 1, QB], F32, tag="acc")
                    sum_acc = sumpool.tile([D + 1, QB], F32, tag="acc2")
                    equads = []  # e tiles of the in-flight group of 4 k-chunks
                    for k in range(N_KT):
                        # PSUM bank limit: matmul N<=512 fp32 out, so each
                        # QB-wide tile is produced/consumed by the tensor
                        # engine in MMN-column slices while ACT/DVE see the
                        # full QB width in one instruction.
                        st = stpool.tile([KT, QB], F32, tag="st")
                        kth = kt_halves[k // (N_KT // 2)]
                        kk = k % (N_KT // 2)
                        for s in range(QB // MMN):
                            nc.tensor.matmul(
                                st[:, s * MMN : (s + 1) * MMN],
                                kth[:, kk * KT : (kk + 1) * KT],
                                qt_sb[:, s * MMN : (s + 1) * MMN],
                                start=True,
                                stop=True,
                            )
                        e = epool.tile([KT, QB], BF16, tag="e")
                        nc.scalar.activation(
                            e[:], st[:], mybir.ActivationFunctionType.Exp, scale=0.125
                        )
                        if vt_sb is None:
                            # V' = [V | ones] per k-chunk, loaded after the
                            # first score tile's inputs are in flight but
                            # ahead of the mask tiles in the DMA ring (mm2
                            # needs it before the first mask consumer runs)
                            vt_sb = vpool.tile([KT, N_KT, D + 1], BF16, tag="vt")
                            nc.sync.dma_start(
                                vt_sb[:, :, 0:D],
                                v[h].rearrange("(n p) d -> p n d", p=KT),
                            )
                            nc.gpsimd.memset(vt_sb[:, :, D : D + 1], 1.0)
                        m = mpool.tile([KT, QB], BF16, tag="m")
                        nc.sync.dma_start(m[:], mask[h, k, qb])
                        em = empool.tile([KT, QB], BF16, tag="em")
                        nc.vector.tensor_mul(em[:], e[:], m[:])
                        for s in range(QB // MMN):
                            nc.tensor.matmul(
                                out_acc[:, s * MMN : (s + 1) * MMN],
                                vt_sb[:, k, :],
                                em[:, s * MMN : (s + 1) * MMN],
                                start=(k == 0),
                                stop=(k == N_KT - 1),
                            )
                        equads.append(e)
                        if len(equads) == 4:
                            # Softmax denominator: instead of a 3rd full PE
                            # stream (ones-row matmul per k-chunk), tree-add
                            # the 4 exp tiles on the DVE (bf16 2x mode, spare
                            # capacity) and feed one quad matmul that rides
                            # the ALREADY-LOADED V' weights of this k — its
                            # ones row accumulates the exact unmasked sums
                            # (rows 0..63 are junk); no extra weight swap.
                            ep0 = epool.tile([KT, QB], BF16, tag="ep")
                            nc.vector.tensor_add(ep0[:], equads[0][:], equads[1][:])
                            ep1 = epool.tile([KT, QB], BF16, tag="ep")
                            nc.vector.tensor_add(ep1[:], equads[2][:], equads[3][:])
                            eq = epool.tile([KT, QB], BF16, tag="eq")
                            nc.vector.tensor_add(eq[:], ep0[:], ep1[:])
                            j = k // 4
                            for s in range(QB // MMN):
                                nc.tensor.matmul(
                                    sum_acc[:, s * MMN : (s + 1) * MMN],
                                    vt_sb[:, k, :],
                                    eq[:, s * MMN : (s + 1) * MMN],
                                    start=(j == 0),
                                    stop=(j == N_KT // 4 - 1),
                                )
                            equads = []
                    out_sb = opool.tile([D, QB], F32, tag="out")
                    nc.vector.tensor_copy(out_sb[:], out_acc[0:D, :])
                    # PSUM reads must start at partition 0 (offset-64 reads
                    # crash the device), so copy the full [65,QB] accumulator
                    # (same cost — engine time scales with free size) and DMA
                    # the sums row out of SBUF instead. On ScalarE so both
                    # accumulator drains run in parallel (shorter PE stall at
                    # the qb boundary — PSUM accumulators have no double
                    # buffer to spare).
                    sums_sb = opool.tile([D + 1, QB], F32, tag="sums")
                    nc.vector.tensor_copy(sums_sb[:], sum_acc[:])
                    nc.sync.dma_start(outu[h, :, qb * QB : (qb + 1) * QB], out_sb[:])
                    nc.sync.dma_start(
                        sums[h : h + 1, qb * QB : (qb + 1) * QB],
                        sums_sb[D : D + 1, :],
                    )

    nc.finalize()
    return nc


_NC_CACHE = None


def _get_nc():
    global _NC_CACHE
    if _NC_CACHE is None:
        _NC_CACHE = build_program()
    return _NC_CACHE


def _dropout_mask_t():
    """keep mask from the reference's fixed jax key, transposed to [bh,k,q]
    and pre-tiled to [bh, k-chunk, qb, 128, QB] bf16 {0,1} so each device
    tile is one contiguous 256KB DMA. Computed on CPU; threefry is
    platform-deterministic."""
    import jax

    cpu = jax.devices("cpu")[0]
    with jax.default_device(cpu):
        keep = jax.random.bernoulli(jax.random.key(42), 1.0 - DROP_P, (B, H, S, S))
        mask_t = jax.numpy.transpose(keep, (0, 1, 3, 2)).astype(jax.numpy.bfloat16)
        mask_t = jax.numpy.reshape(mask_t, (B * H, N_KT, KT, N_QB, QB))
        mask_t = jax.numpy.transpose(mask_t, (0, 1, 3, 2, 4))
        mask_t = np.ascontiguousarray(np.asarray(mask_t))
    return mask_t


def kernel(x1: np.ndarray, x2: np.ndarray, _trace: bool = False) -> np.ndarray:
    global LAST_RESULTS
    nc = _get_nc()

    bh = B * H
    x1f = np.asarray(x1, dtype=np.float32).reshape(bh, S, D)
    x2f = np.asarray(x2, dtype=np.float32).reshape(bh, S, D)
    qt_all = np.ascontiguousarray(x1f.transpose(0, 2, 1)).astype(ml_dtypes.bfloat16)
    kt_all = np.ascontiguousarray(x2f.transpose(0, 2, 1)).astype(ml_dtypes.bfloat16)
    v_all = x2f.astype(ml_dtypes.bfloat16)
    mask_all = _dropout_mask_t()

    in_maps = []
    for c in range(N_CORES):
        sl = slice(c * HEADS_PER_CORE, (c + 1) * HEADS_PER_CORE)
        in_maps.append(
            {
                "qt": qt_all[sl],
                "kt": kt_all[sl],
                "v": v_all[sl],
                "mask": mask_all[sl],
            }
        )

    res = run_bass_kernel_spmd(nc, in_maps, core_ids=list(range(N_CORES)), trace=_trace)
    LAST_RESULTS = res

    outu = np.concatenate([r["outu"] for r in res.results], axis=0)  # [32, D, S]
    sums = np.concatenate([r["sums"] for r in res.results], axis=0)  # [32, S]

    denom = (1.0 - DROP_P) * sums  # [32, S] (per q)
    out = outu / denom[:, None, :]  # [32, D, S]
    out = out.transpose(0, 2, 1).reshape(B, H, S, D)
    return np.ascontiguousarray(out.astype(np.float32))


# revision 19
# speedup vs baseline: 1.2356x; 1.0002x over previous
"""Multi-head attention with dropout on 8 Trainium2 NeuronCores.

Problem: B=2, H=16, S=2048, D=64 attention where x2 serves as both keys and
values, softmax over keys, then a deterministic jax dropout mask (key 42,
p=0.1) applied to the probabilities before the value matmul.

Sharding: the 32 (b,h) pairs are split 4-per-core across 8 cores (pure head
parallelism, no collectives).

Device formulation (per head): scores are computed TRANSPOSED, St[k,q] =
sum_d K[k,d]Q[q,d], by matmul(lhsT=K^T chunk, rhs=Q^T). Softmax max-
subtraction is skipped (scores ~ N(0,1), exp is safe in fp32). The exp runs
on the scalar engine with the 1/sqrt(64) fold into its scale. The dropout
mask (shipped from host as bf16 {0,1}, transposed to [k,q]) is applied by
the vector engine. The second matmul uses V'=[V|ones] as stationary weights
twice per k-chunk: once against masked exp (rows 0..63 accumulate the
unnormalized output, transposed [d,q]) and once against unmasked exp (row 64
accumulates the softmax denominator). Normalization by 1/(0.9*sum) and the
final [d,q]->[q,d] transpose happen on host during the gather.
"""

import sys

sys.path.insert(0, "/opt/trn_rl_repo")

import numpy as np
import ml_dtypes

import concourse.bass as bass
import concourse.mybir as mybir
import concourse.tile as tile
from concourse import bacc
from concourse.bass_utils import run_bass_kernel_spmd

B, H, S, D = 2, 16, 2048, 64
N_CORES = 8
HEADS_PER_CORE = (B * H) // N_CORES  # 4
DROP_P = 0.1

QB = 1024  # q columns per activation/vector instruction
MMN = 512  # q columns per matmul instruction (PSUM bank limit, fp32 out)
KT = 128  # k rows per score tile (matmul output partitions)
N_KT = S // KT  # 16
N_QB = S // QB  # 2

BF16 = mybir.dt.bfloat16
F32 = mybir.dt.float32

LAST_RESULTS = None  # BassKernelResults of the most recent run (for test.py)


def build_program():
    nc = bacc.Bacc()

    qt = nc.dram_tensor("qt", [HEADS_PER_CORE, D, S], BF16, kind="ExternalInput")
    kt = nc.dram_tensor("kt", [HEADS_PER_CORE, D, S], BF16, kind="ExternalInput")
    v = nc.dram_tensor("v", [HEADS_PER_CORE, S, D], BF16, kind="ExternalInput")
    # mask pre-tiled on host: [h, k-chunk, qb, 128, QB] so each tile DMA is
    # one fully contiguous 256KB read
    mask = nc.dram_tensor(
        "mask", [HEADS_PER_CORE, N_KT, N_QB, KT, QB], BF16, kind="ExternalInput"
    )
    outu = nc.dram_tensor("outu", [HEADS_PER_CORE, D, S], F32, kind="ExternalOutput")
    sums = nc.dram_tensor("sums", [HEADS_PER_CORE, S], F32, kind="ExternalOutput")

    with tile.TileContext(nc) as tc:
        with (
            tc.tile_pool(name="qkpool", bufs=2) as qkpool,
            tc.tile_pool(name="vpool", bufs=2) as vpool,
            tc.tile_pool(name="epool", bufs=6) as epool,
            tc.tile_pool(name="empool", bufs=6) as empool,
            tc.tile_pool(name="mpool", bufs=12) as mpool,
            tc.tile_pool(name="opool", bufs=2) as opool,
            tc.tile_pool(name="stpool", bufs=2, space=bass.MemorySpace.PSUM) as stpool,
            tc.tile_pool(name="accpool", bufs=1, space=bass.MemorySpace.PSUM) as accpool,
            tc.tile_pool(name="sumpool", bufs=1, space=bass.MemorySpace.PSUM) as sumpool,
        ):
            # Warmup: the first ACTIVATE pays a ~2.7us ACT_TABLE_LOAD; fire a
            # throwaway exp immediately so the table loads while the initial
            # DMAs are still in flight instead of on the first tile's
            # critical path.
            warm_in = opool.tile([KT, 16], F32, tag="warm_in")
            nc.gpsimd.memset(warm_in[:], 0.0)
            warm_out = opool.tile([KT, 16], BF16, tag="warm_out")
            nc.scalar.activation(
                warm_out[:], warm_in[:], mybir.ActivationFunctionType.Exp
            )

            for h in range(HEADS_PER_CORE):
                # K^T split in halves and Q^T loaded per qb so the first
                # matmuls only wait on ~0.5MB of DMA, not the whole head's
                # inputs (tile deps are whole-tile).
                kt_halves = []
                for kh in range(2):
                    kt_sb = qkpool.tile([D, S // 2], BF16, tag=f"kt{kh}")
                    nc.sync.dma_start(kt_sb[:], kt[h][:, kh * (S // 2) : (kh + 1) * (S // 2)])
                    kt_halves.append(kt_sb)
                vt_sb = None

                for qb in range(N_QB):
                    qt_sb = qkpool.tile([D, QB], BF16, tag="qt")
                    nc.sync.dma_start(qt_sb[:], qt[h][:, qb * QB : (qb + 1) * QB])
                    out_acc = accpool.tile([D + 1, QB], F32, tag="acc")
                    sum_acc = sumpool.tile([D + 1, QB], F32, tag="acc2")
                    equads = []  # e tiles of the in-flight group of 4 k-chunks
                    for k in range(N_KT):
                        # PSUM bank limit: matmul N<=512 fp32 out, so each
                        # QB-wide tile is produced/consumed by the tensor
                        # engine in MMN-column slices while ACT/DVE see the
                        # full QB width in one instruction.
                        st = stpool.tile([KT, QB], F32, tag="st")
                        kth = kt_halves[k // (N_KT // 2)]
                        kk = k % (N_KT // 2)
                        for s in range(QB // MMN):
                            nc.tensor.matmul(
                                st[:, s * MMN : (s + 1) * MMN],
                                kth[:, kk * KT : (kk + 1) * KT],
                                qt_sb[:, s * MMN : (s + 1) * MMN],
                                start=True,
                                stop=True,
                            )
                        e = epool.tile([KT, QB], BF16, tag="e")
                        nc.scalar.activation(
                            e[:], st[:], mybir.ActivationFunctionType.Exp, scale=0.125
                        )
                        if vt_sb is None:
                            # V' = [V | ones] per k-chunk, loaded after the
                            # first score tile's inputs are in flight but
                            # ahead of the mask tiles in the DMA ring (mm2
                            # needs it before the first mask consumer runs)
                            vt_sb = vpool.tile([KT, N_KT, D + 1], BF16, tag="vt")
                            nc.sync.dma_start(
                                vt_sb[:, :, 0:D],
                                v[h].rearrange("(n p) d -> p n d", p=KT),
                            )
                            nc.gpsimd.memset(vt_sb[:, :, D : D + 1], 1.0)
                        m = mpool.tile([KT, QB], BF16, tag="m")
                        nc.sync.dma_start(m[:], mask[h, k, qb])
                        em = empool.tile([KT, QB], BF16, tag="em")
                        nc.vector.tensor_mul(em[:], e[:], m[:])
                        for s in range(QB // MMN):
                            nc.tensor.matmul(
                                out_acc[:, s * MMN : (s + 1) * MMN],
                                vt_sb[:, k, :],
                                em[:, s * MMN : (s + 1) * MMN],
                                start=(k == 0),
                                stop=(k == N_KT - 1),
                            )
                        equads.append(e)
                        if len(equads) == 4:
                            # Softmax denominator: instead of a 3rd full PE
                            # stream (ones-row matmul per k-chunk), tree-add
                            # the 4 exp tiles on the DVE (bf16 2x mode, spare
                            # capacity) and feed one quad matmul that rides
                            # the ALREADY-LOADED V' weights of this k — its
                            # ones row accumulates the exact unmasked sums
                            # (rows 0..63 are junk); no extra weight swap.
                            ep0 = epool.tile([KT, QB], BF16, tag="ep")
                            nc.vector.tensor_add(ep0[:], equads[0][:], equads[1][:])
                            ep1 = epool.tile([KT, QB], BF16, tag="ep")
                            nc.vector.tensor_add(ep1[:], equads[2][:], equads[3][:])
                            eq = epool.tile([KT, QB], BF16, tag="eq")
                            nc.vector.tensor_add(eq[:], ep0[:], ep1[:])
                            j = k // 4
                            for s in range(QB // MMN):
                                nc.tensor.matmul(
                                    sum_acc[:, s * MMN : (s + 1) * MMN],
                                    vt_sb[:, k, :],
                                    eq[:, s * MMN : (s + 1) * MMN],
                                    start=(j == 0),
                                    stop=(j == N_KT // 4 - 1),
                                )
                            equads = []
                    out_sb = opool.tile([D, QB], F32, tag="out")
                    nc.vector.tensor_copy(out_sb[:], out_acc[0:D, :])
                    # PSUM reads must start at partition 0 (offset-64 reads
                    # crash the device), so copy the full [65,QB] accumulator
                    # (same cost — engine time scales with free size) and DMA
                    # the sums row out of SBUF instead. On ScalarE so both
                    # accumulator drains run in parallel (shorter PE stall at
                    # the qb boundary — PSUM accumulators have no double
                    # buffer to spare).
                    sums_sb = opool.tile([D + 1, QB], F32, tag="sums")
                    nc.vector.tensor_copy(sums_sb[:], sum_acc[:])
                    nc.sync.dma_start(outu[h, :, qb * QB : (qb + 1) * QB], out_sb[:])
                    nc.sync.dma_start(
                        sums[h : h + 1, qb * QB : (qb + 1) * QB],
                        sums_sb[D : D + 1, :],
                    )

    nc.finalize()
    return nc


_NC_CACHE = None


def _get_nc():
    global _NC_CACHE
    if _NC_CACHE is None:
        _NC_CACHE = build_program()
    return _NC_CACHE


def _dropout_mask_t():
    """keep mask from the reference's fixed jax key, transposed to [bh,k,q]
    and pre-tiled to [bh, k-chunk, qb, 128, QB] bf16 {0,1} so each device
    tile is one contiguous 256KB DMA. Computed on CPU; threefry is
    platform-deterministic."""
    import jax

    cpu = jax.devices("cpu")[0]
    with jax.default_device(cpu):
        keep = jax.random.bernoulli(jax.random.key(42), 1.0 - DROP_P, (B, H, S, S))
        mask_t = jax.numpy.transpose(keep, (0, 1, 3, 2)).astype(jax.numpy.bfloat16)
        mask_t = jax.numpy.reshape(mask_t, (B * H, N_KT, KT, N_QB, QB))
        mask_t = jax.numpy.transpose(mask_t, (0, 1, 3, 2, 4))
        mask_t = np.ascontiguousarray(np.asarray(mask_t))
    return mask_t


def kernel(x1: np.ndarray, x2: np.ndarray, _trace: bool = False) -> np.ndarray:
    global LAST_RESULTS
    nc = _get_nc()

    bh = B * H
    x1f = np.asarray(x1, dtype=np.float32).reshape(bh, S, D)
    x2f = np.asarray(x2, dtype=np.float32).reshape(bh, S, D)
    qt_all = np.ascontiguousarray(x1f.transpose(0, 2, 1)).astype(ml_dtypes.bfloat16)
    kt_all = np.ascontiguousarray(x2f.transpose(0, 2, 1)).astype(ml_dtypes.bfloat16)
    v_all = x2f.astype(ml_dtypes.bfloat16)
    mask_all = _dropout_mask_t()

    in_maps = []
    for c in range(N_CORES):
        sl = slice(c * HEADS_PER_CORE, (c + 1) * HEADS_PER_CORE)
        in_maps.append(
            {
                "qt": qt_all[sl],
                "kt": kt_all[sl],
                "v": v_all[sl],
                "mask": mask_all[sl],
            }
        )

    res = run_bass_kernel_spmd(nc, in_maps, core_ids=list(range(N_CORES)), trace=_trace)
    LAST_RESULTS = res

    outu = np.concatenate([r["outu"] for r in res.results], axis=0)  # [32, D, S]
    sums = np.concatenate([r["sums"] for r in res.results], axis=0)  # [32, S]

    denom = (1.0 - DROP_P) * sums  # [32, S] (per q)
    out = outu / denom[:, None, :]  # [32, D, S]
    out = out.transpose(0, 2, 1).reshape(B, H, S, D)
    return np.ascontiguousarray(out.astype(np.float32))
